# revision 1
# baseline (speedup 1.0000x reference)
"""AdaptiveGraphLearning kernel for 8 TRN2 NeuronCores.

w[i,j] = sigmoid(sum_h W2[h]*relu(A[i,h]+B[j,h]) + b2), strict upper triangle,
thresholded at 0.1. Only the upper triangle is computed (2x saving) with a
balanced row split: core c owns row blocks [128c,128c+128) and
[2048-128(c+1), 2048-128c) -- exactly 5 [128,512] output supertiles per core.

Device hot loop per supertile: for each h-chunk (4 h's) x col-group (32 i's):
  R = relu(Brep + Acol) on DVE/ACT (tensor_scalar / activation, f32r out)
  matmul(psum[32b:32b+32], lhsT=S_chunk, rhs=R) accumulating over 16 chunks
K=128 packs 32 i's x 4 h's; S_chunk is block-diagonal with W2 values, so the
H-reduction runs on the TensorEngine at 1 col/cycle (float32r).
Raw bass with explicit semaphores (hardware allows 1 sem-wait per instr).
"""
import numpy as np

N, F, H = 2048, 256, 64
P, JC, HC, G, NB, NCH = 128, 512, 4, 32, 4, 16
NCORES = 8
NST = 5          # supertiles per core
NSLOT = 4        # brep j-span slots (st 3,4 share slot 3)
NRING = 11        # R-tile ring size
THRESH = 0.1


def _core_layout(c):
    """Return (i_top, i_bot, sts) where sts is a list of (block, chunk)."""
    i_top = 128 * c
    i_bot = N - 128 * (c + 1)
    tops = [(0, j) for j in range(i_top // 512, 4)]
    bots = [(1, j) for j in range(i_bot // 512, 4)]
    sts = [x for x in tops if x != (0, 3)] + [x for x in bots if x != (1, 3)]
    sts = sts + [(0, 3), (1, 3)]
    assert len(sts) == NST, (c, sts)
    return i_top, i_bot, sts


def _build_core_inputs(c, A, BT, W2, b2):
    """Host-side layout prep for one core (pure reformatting of A/B/W2)."""
    i_top, i_bot, sts = _core_layout(c)
    pg = np.arange(P) % G          # g = p % 32
    ph = np.arange(P) // G         # hc = p // 32

    # brep[p, ch, slot, :] = BT[4*ch + p//32, jspan(slot)]
    brep = np.empty((P, NCH, NSLOT, JC), np.float32)
    for s in range(NSLOT):
        blk, ch_j = sts[s]
        j0 = 512 * ch_j
        for chk in range(NCH):
            brep[:, chk, s, :] = BT[4 * chk + ph][:, j0:j0 + JC]

    # acol[p, st, b, ch] = A[iblk(st) + 32b + p%32, 4*ch + p//32]
    acol = np.empty((P, NST, NB, NCH), np.float32)
    tri = np.empty((P, NST, JC), np.float32)
    for st, (blk, ch_j) in enumerate(sts):
        iblk = i_top if blk == 0 else i_bot
        j0 = 512 * ch_j
        for b in range(NB):
            rows = iblk + 32 * b + pg          # [128]
            cols = 4 * np.arange(NCH)[None, :] + ph[:, None]   # [128, 16]
            acol[:, st, b, :] = A[rows[:, None], cols]
        jj = j0 + np.arange(JC)[None, :]
        ii = (iblk + np.arange(P))[:, None]
        tri[:, st, :] = (jj > ii).astype(np.float32)

    s_mat = np.zeros((P, NCH, G), np.float32)
    for chk in range(NCH):
        s_mat[np.arange(P), chk, pg] = W2[4 * chk + ph]

    b2bc = np.full((P, 1), b2, np.float32)
    misc = np.concatenate(
        [acol.reshape(P, -1), s_mat.reshape(P, -1), tri.reshape(P, -1), b2bc],
        axis=1).astype(np.float32).copy()
    return brep.reshape(P, -1).copy(), misc


def _build_graph(nc_mod, bass, mybir):
    nc = bass.Bass()
    MF_ACOL = NST * NB * NCH
    MF_S = NCH * G
    MF_TRI = NST * JC
    MF = MF_ACOL + MF_S + MF_TRI + 1

    brep_e = nc.declare_dram_parameter("brep", [P, NCH * NSLOT * JC], mybir.dt.float32, isOutput=False)
    misc_e = nc.declare_dram_parameter("misc", [P, MF], mybir.dt.float32, isOutput=False)
    w_e = nc.declare_dram_parameter("w_out", [NST * P, JC], mybir.dt.float32, isOutput=True)
    m_e = nc.declare_dram_parameter("m_out", [NST * P, JC], mybir.dt.uint8, isOutput=True)

    brep_sb = nc.alloc_sbuf_tensor("brep_sb", [P, NCH, NSLOT, JC], mybir.dt.float32)
    misc_sb = nc.alloc_sbuf_tensor("misc_sb", [P, MF], mybir.dt.float32)
    s_full = nc.alloc_sbuf_tensor("s_full", [P, NCH, NB, P], mybir.dt.float32r)
    ring = nc.alloc_sbuf_tensor("ring", [P, NRING, JC], mybir.dt.float32r)
    w_buf = nc.alloc_sbuf_tensor("w_buf", [P, 2, JC], mybir.dt.float32)
    wm_buf = nc.alloc_sbuf_tensor("wm_buf", [P, 2, JC], mybir.dt.float32)
    mf_buf = nc.alloc_sbuf_tensor("mf_buf", [P, JC], mybir.dt.float32)
    m8_buf = nc.alloc_sbuf_tensor("m8_buf", [P, 2, JC], mybir.dt.uint8)
    z1 = nc.alloc_sbuf_tensor("z1", [P, 1], mybir.dt.float32)
    ps = [nc.alloc_psum_tensor(f"ps{st}", [P, JC], mybir.dt.float32) for st in range(NST)]

    def ap(h):
        return h.ap() if hasattr(h, "ap") else h

    misc = ap(misc_sb)
    acol_v = misc[:, :MF_ACOL].rearrange("p (st b ch) -> p st b ch", st=NST, b=NB)
    s_f32 = misc[:, MF_ACOL:MF_ACOL + MF_S]
    tri_v = misc[:, MF_ACOL + MF_S:MF_ACOL + MF_S + MF_TRI].rearrange(
        "p (st j) -> p st j", st=NST)
    b2_v = misc[:, MF - 1:MF]

    # global gen/MM schedule
    sched = []
    for chk in range(NCH):
        for st in range(NST):
            for b in range(NB):
                k = len(sched)
                sched.append((k, chk, st, b,
                              "a" if k % 14 in (2, 5, 8, 11, 13) else "v"))
    ndve_le = np.cumsum([1 if e == "v" else 0 for (_, _, _, _, e) in sched])
    nact_le = np.cumsum([1 if e == "a" else 0 for (_, _, _, _, e) in sched])
    last_mm = {st: max(k for (k, c2, s2, b2_, e) in sched if s2 == st and c2 == NCH - 1)
               for st in range(NST)}
    first_of_chunk = {}
    for (k, chk, st, b, e) in sched:
        key = (e, chk)
        if key not in first_of_chunk:
            first_of_chunk[key] = k

    with (nc.Block() as block,
          nc.semaphore("s_in") as s_in,
          nc.semaphore("g_dve") as g_dve,
          nc.semaphore("g_act") as g_act,
          nc.semaphore("pe") as pe,
          nc.semaphore("a_ep") as a_ep,
          nc.semaphore("d_ep") as d_ep,
          nc.semaphore("s_out") as s_out,
          nc.semaphore("g_sf") as g_sf):

        @block.sync
        def _(sync):
            sync.dma_start(out=misc[:], in_=ap(misc_e)).then_inc(s_in, 16)
            bs = ap(brep_sb)
            be = ap(brep_e).rearrange("p (ch r) -> p ch r", ch=NCH)
            for chk in range(NCH):
                sync.dma_start(out=bs[:, chk], in_=be[:, chk]).then_inc(s_in, 16)
            for st in range(NST):
                sync.wait_ge(d_ep, st + 1)
                sync.dma_start(out=ap(w_e)[st * P:(st + 1) * P, :],
                               in_=ap(wm_buf)[:, st % 2, :]).then_inc(s_out, 16)
                sync.dma_start(out=ap(m_e)[st * P:(st + 1) * P, :],
                               in_=ap(m8_buf)[:, st % 2, :]).then_inc(s_out, 16)
            sync.wait_ge(s_out, 32 * NST)

        @block.tensor
        def _(tensor):
            for (k, chk, st, b, e) in sched:
                if k % 80 == 0:
                    tensor.wait_ge(g_sf, chk + 1)
                if e == "v":
                    tensor.wait_ge(g_dve, int(ndve_le[k]))
                else:
                    tensor.wait_ge(g_act, int(nact_le[k]))
                tensor.matmul(ap(ps[st])[:, :],
                              lhsT=ap(s_full)[:, chk, b, :],
                              rhs=ap(ring)[:, k % NRING, :],
                              start=(chk == 0 and b == 0),
                              stop=(chk == NCH - 1 and b == NB - 1),
                              skip_group_check=True).then_inc(pe, 1)

        @block.vector
        def _(vector):
            def dve_epilogue(st):
                vector.wait_ge(a_ep, st + 1)
                if st >= 2:
                    vector.wait_ge(s_out, 32 * (st - 1))
                wmv = ap(wm_buf)[:, st % 2, :]
                vector.tensor_tensor(wmv, ap(w_buf)[:, st % 2, :], tri_v[:, st, :],
                                     op=mybir.AluOpType.mult)
                vector.tensor_scalar(ap(mf_buf)[:], wmv, THRESH, None,
                                     op0=mybir.AluOpType.is_gt)
                vector.tensor_tensor(wmv, wmv, ap(mf_buf)[:],
                                     op=mybir.AluOpType.mult)
                vector.tensor_copy(ap(m8_buf)[:, st % 2, :],
                                   ap(mf_buf)[:]).then_inc(d_ep, 1)

            s3 = s_f32.rearrange("p (ch g) -> p ch g", ch=NCH)
            sfv = ap(s_full)
            vector.wait_ge(s_in, 16)
            vector.memset(ap(z1), 0.0)
            vector.tensor_copy(sfv.rearrange("p a b c -> p (a b c)"),
                               ap(z1).to_broadcast((P, NCH * NB * P)))
            done_sf = set()
            for (k, chk, st, b, e) in sched:
                if chk not in done_sf:
                    done_sf.add(chk)
                    vector.wait_ge(s_in, 16 * (chk + 2))
                    for bb in range(NB):
                        vector.tensor_copy(sfv[:, chk, bb, 32 * bb:32 * (bb + 1)],
                                           s3[:, chk, :])
                    vector.sem_inc(g_sf, 1)
                if e == "v":
                    if first_of_chunk[("v", chk)] == k:
                        vector.wait_ge(s_in, 16 * (chk + 2))
                    if k >= NRING:
                        vector.wait_ge(pe, k - NRING + 1)
                    vector.tensor_scalar(
                        ap(ring)[:, k % NRING, :],
                        ap(brep_sb)[:, chk, min(st, 3), :],
                        acol_v[:, st, b, :][:, chk:chk + 1], 0.0,
                        op0=mybir.AluOpType.add,
                        op1=mybir.AluOpType.max).then_inc(g_dve, 1)
                if chk == NCH - 1 and b == NB - 1:
                    dve_epilogue(st)

        @block.scalar
        def _(scalar):
            for (k, chk, st, b, e) in sched:
                if e == "a":
                    if first_of_chunk[("a", chk)] == k:
                        scalar.wait_ge(s_in, 16 * (chk + 2))
                    if k >= NRING:
                        scalar.wait_ge(pe, k - NRING + 1)
                    scalar.activation(
                        ap(ring)[:, k % NRING, :],
                        ap(brep_sb)[:, chk, min(st, 3), :],
                        mybir.ActivationFunctionType.Relu,
                        bias=acol_v[:, st, b, :][:, chk:chk + 1],
                        scale=1.0).then_inc(g_act, 1)
                if chk == NCH - 1 and b == NB - 1:
                    scalar.wait_ge(pe, last_mm[st] + 1)
                    if st >= 2:
                        scalar.wait_ge(s_out, 32 * (st - 1))
                    scalar.activation(ap(w_buf)[:, st % 2, :], ap(ps[st])[:, :],
                                      mybir.ActivationFunctionType.Sigmoid,
                                      bias=b2_v, scale=1.0).then_inc(a_ep, 1)

    return nc


def kernel(node_features, node_emb, W_fe, b_fe, W1, b1, W2, b2):
    import concourse.bass as bass
    import concourse.mybir as mybir
    from concourse.bass_utils import run_bass_kernel_spmd

    nf = np.asarray(node_features, np.float32)
    emb = np.asarray(node_emb, np.float32)
    W_fe = np.asarray(W_fe, np.float32)
    b_fe = np.asarray(b_fe, np.float32)
    W1 = np.asarray(W1, np.float32)
    b1 = np.asarray(b1, np.float32)
    W2v = np.asarray(W2, np.float32)[0]
    b2v = float(np.asarray(b2, np.float32)[0])

    comb = nf @ W_fe.T + b_fe + emb
    A = (comb @ W1[:, :H].T).astype(np.float32)
    BT = (comb @ W1[:, H:].T + b1).astype(np.float32).T.copy()

    nc = _build_graph(None, bass, mybir)
    in_maps = []
    for c in range(NCORES):
        brep, misc = _build_core_inputs(c, A, BT, W2v, b2v)
        in_maps.append({"brep": brep, "misc": misc})
    res = run_bass_kernel_spmd(nc, in_maps, core_ids=list(range(NCORES)))

    full_w = np.zeros((N, N), np.float32)
    full_m = np.zeros((N, N), bool)
    for c in range(NCORES):
        i_top, i_bot, sts = _core_layout(c)
        wo = res.results[c]["w_out"].reshape(NST, P, JC)
        mo = res.results[c]["m_out"].reshape(NST, P, JC)
        for st, (blk, ch_j) in enumerate(sts):
            iblk = i_top if blk == 0 else i_bot
            j0 = 512 * ch_j
            full_w[iblk:iblk + P, j0:j0 + JC] = wo[st]
            full_m[iblk:iblk + P, j0:j0 + JC] = mo[st].astype(bool)
    return full_w, full_m



# revision 4
# speedup vs baseline: 1.0790x; 1.0790x over previous
"""AdaptiveGraphLearning kernel for 8 TRN2 NeuronCores (v2a, fp16 gen+matmul).

w[i,j] = sigmoid(sum_h W2[h]*relu(A[i,h]+B[j,h]) + b2), strict upper triangle,
thresholded at 0.1. Only the upper triangle is computed (2x saving) with a
balanced row split: core c owns row blocks [128c,128c+128) and
[2048-128(c+1), 2048-128c) -- exactly 5 [128,512] output supertiles per core.

v2a design:
 - A,B prescaled by |W2| on host so the reduction selector S is +-1 and the
   W2 multiply costs nothing on device; logit = sum_h sign(W2h)*relu(A'+B').
 - fp16 brep/ring/S: DVE tensor_scalar runs in 2x/4x perf mode, matmul
   moving data is fp16 (1 col/cycle), input DMA is ~5MB instead of 16MB.
 - G=8 packing: K=128 packs 8 i's x 16 h's, so brep is only replicated 8x.
 - st-outer schedule: each supertile's PSUM finishes right after its own
   64 matmuls, so sigmoid/threshold/mask/DMA overlap the next supertile;
   epilogues are issued a few gen ops into the next supertile so the gen
   engines don't stall on the PE ring drain.
 - epilogue ops and semaphore style are the baseline's (per-op incs).
"""
import numpy as np

N, F, H = 2048, 256, 64
P, JC = 128, 512
G = 8                 # i's per matmul band
HC = 16               # h's per chunk
NCH = H // HC         # 4 chunks
NB = P // G           # 16 col-groups
NCORES = 8
NST = 5               # supertiles per core
NSLOT = 4             # brep j-span slots (st 3,4 share slot 3)
NRING = 12            # R-tile ring size
NMM = NST * NCH * NB  # 320 matmuls / gen ops
THRESH = 0.1


def _core_layout(c):
    """Return (i_top, i_bot, sts) where sts is a list of (block, chunk)."""
    i_top = 128 * c
    i_bot = N - 128 * (c + 1)
    tops = [(0, j) for j in range(i_top // 512, 4)]
    bots = [(1, j) for j in range(i_bot // 512, 4)]
    sts = [x for x in tops if x != (0, 3)] + [x for x in bots if x != (1, 3)]
    sts = sts + [(0, 3), (1, 3)]
    assert len(sts) == NST, (c, sts)
    return i_top, i_bot, sts


def _schedule():
    """Global op order: st outer, then chunk, then band. Returns list of
    (k, st, chk, b, eng) with eng in {'v','a'}."""
    sched = []
    for st in range(NST):
        for chk in range(NCH):
            for b in range(NB):
                k = len(sched)
                sched.append((k, st, chk, b, "a" if k % 5 == 2 else "v"))
    return sched


def _build_core_inputs(c, A, BT, sgn, b2):
    """Host-side layout prep for one core (pure reformatting of A'/B'/sign).

    A is prescaled A*|W2| [N,H] f32; BT is prescaled (B*|W2|).T [H,N] f32;
    sgn is sign(W2) [H]."""
    i_top, i_bot, sts = _core_layout(c)
    pg = np.arange(P) % G          # g  = p % 8   -> i offset within band
    ph = np.arange(P) // G         # hc = p // 8  -> h within chunk

    # brep[p, ch, slot, :] = BT[HC*ch + p//G, jspan(slot)]   (fp16)
    brep = np.empty((P, NCH, NSLOT, JC), np.float16)
    for s in range(NSLOT):
        blk, ch_j = sts[s]
        j0 = 512 * ch_j
        for chk in range(NCH):
            brep[:, chk, s, :] = BT[HC * chk + ph][:, j0:j0 + JC]

    # acol[p, st, b, ch] = A[iblk(st) + G*b + p%G, HC*ch + p//G]   (f32)
    acol = np.empty((P, NST, NB, NCH), np.float32)
    tri = np.empty((P, NST, JC), np.float32)
    for st, (blk, ch_j) in enumerate(sts):
        iblk = i_top if blk == 0 else i_bot
        j0 = 512 * ch_j
        for b in range(NB):
            rows = iblk + G * b + pg           # [128]
            cols = HC * np.arange(NCH)[None, :] + ph[:, None]   # [128, NCH]
            acol[:, st, b, :] = A[rows[:, None], cols]
        jj = j0 + np.arange(JC)[None, :]
        ii = (iblk + np.arange(P))[:, None]
        tri[:, st, :] = (jj > ii).astype(np.float32)

    # s_full[p, chk, b, G*b + p%G] = sign(W2[HC*chk + p//G])   (fp16)
    s_full = np.zeros((P, NCH, NB, P), np.float16)
    for chk in range(NCH):
        for b in range(NB):
            s_full[np.arange(P), chk, b, G * b + pg] = sgn[HC * chk + ph]

    b2bc = np.full((P, 1), b2, np.float32)
    misc = np.concatenate(
        [acol.reshape(P, -1), tri.reshape(P, -1), b2bc], axis=1).astype(
        np.float32).copy()
    return {"brep": brep.reshape(P, -1).copy(),
            "aux": s_full.reshape(P, -1).copy(), "misc": misc}


def _build_graph(bass, mybir):
    nc = bass.Bass()
    MF_ACOL = NST * NB * NCH
    MF_TRI = NST * JC
    MF = MF_ACOL + MF_TRI + 1
    SF = NCH * NB * P

    brep_e = nc.declare_dram_parameter("brep", [P, NCH * NSLOT * JC], mybir.dt.float16, isOutput=False)
    aux_e = nc.declare_dram_parameter("aux", [P, SF], mybir.dt.float16, isOutput=False)
    misc_e = nc.declare_dram_parameter("misc", [P, MF], mybir.dt.float32, isOutput=False)
    w_e = nc.declare_dram_parameter("w_out", [NST * P, JC], mybir.dt.float32, isOutput=True)
    m_e = nc.declare_dram_parameter("m_out", [NST * P, JC], mybir.dt.uint8, isOutput=True)

    brep_sb = nc.alloc_sbuf_tensor("brep_sb", [P, NCH, NSLOT, JC], mybir.dt.float16)
    aux_sb = nc.alloc_sbuf_tensor("aux_sb", [P, SF], mybir.dt.float16)
    misc_sb = nc.alloc_sbuf_tensor("misc_sb", [P, MF], mybir.dt.float32)
    ring = nc.alloc_sbuf_tensor("ring", [P, NRING, JC], mybir.dt.float16)
    w_buf = nc.alloc_sbuf_tensor("w_buf", [P, 2, JC], mybir.dt.float32)
    wm_buf = nc.alloc_sbuf_tensor("wm_buf", [P, 2, JC], mybir.dt.float32)
    mf_buf = nc.alloc_sbuf_tensor("mf_buf", [P, JC], mybir.dt.float32)
    m8_buf = nc.alloc_sbuf_tensor("m8_buf", [P, 2, JC], mybir.dt.uint8)
    ps = [nc.alloc_psum_tensor(f"ps{st}", [P, JC], mybir.dt.float32) for st in range(NST)]

    def ap(h):
        return h.ap() if hasattr(h, "ap") else h

    misc = ap(misc_sb)
    acol_v = misc[:, :MF_ACOL].rearrange("p (st b ch) -> p st b ch", st=NST, b=NB)
    tri_v = misc[:, MF_ACOL:MF_ACOL + MF_TRI].rearrange(
        "p (st j) -> p st j", st=NST)
    b2_v = misc[:, MF - 1:MF]
    sfull_v = ap(aux_sb)[:, :].rearrange("p (ch b m) -> p ch b m", ch=NCH, b=NB)

    sched = _schedule()
    ndve_le = np.cumsum([1 if e == "v" else 0 for (_, _, _, _, e) in sched])
    nact_le = np.cumsum([1 if e == "a" else 0 for (_, _, _, _, e) in sched])
    last_mm = {st: 64 * (st + 1) for st in range(NST)}   # pe value when ps[st] done

    first_chunk_use = {}
    for (k, st, chk, b, e) in sched:
        key = (e, chk)
        if key not in first_chunk_use:
            first_chunk_use[key] = k

    with (nc.Block() as block,
          nc.semaphore("s_in") as s_in,
          nc.semaphore("g_dve") as g_dve,
          nc.semaphore("g_act") as g_act,
          nc.semaphore("pe") as pe,
          nc.semaphore("a_ep") as a_ep,
          nc.semaphore("d_ep") as d_ep,
          nc.semaphore("s_out") as s_out):

        @block.sync
        def _(sync):
            sync.dma_start(out=misc[:], in_=ap(misc_e)).then_inc(s_in, 16)
            sync.dma_start(out=ap(aux_sb)[:, :], in_=ap(aux_e)).then_inc(s_in, 16)
            bs = ap(brep_sb)
            be = ap(brep_e).rearrange("p (ch r) -> p ch r", ch=NCH)
            for chk in range(NCH):
                sync.dma_start(out=bs[:, chk], in_=be[:, chk]).then_inc(s_in, 16)
            for st in range(NST):
                sync.wait_ge(d_ep, st + 1)
                sync.dma_start(out=ap(w_e)[st * P:(st + 1) * P, :],
                               in_=ap(wm_buf)[:, st % 2, :]).then_inc(s_out, 16)
                sync.dma_start(out=ap(m_e)[st * P:(st + 1) * P, :],
                               in_=ap(m8_buf)[:, st % 2, :]).then_inc(s_out, 16)
            sync.wait_ge(s_out, 32 * NST)

        @block.tensor
        def _(tensor):
            tensor.wait_ge(s_in, 32)   # aux (s_full) loaded
            for (k, st, chk, b, e) in sched:
                if e == "v":
                    tensor.wait_ge(g_dve, int(ndve_le[k]))
                else:
                    tensor.wait_ge(g_act, int(nact_le[k]))
                tensor.matmul(ap(ps[st])[:, :],
                              lhsT=sfull_v[:, chk, b, :],
                              rhs=ap(ring)[:, k % NRING, :],
                              start=(chk == 0 and b == 0),
                              stop=(chk == NCH - 1 and b == NB - 1),
                              skip_group_check=True).then_inc(pe, 1)

        @block.vector
        def _(vector):
            def v_epilogue(st):
                # issued ~16 gen ops into the next supertile so the PE ring
                # drain has already completed and this doesn't stall gen
                vector.wait_ge(a_ep, st + 1)
                if st >= 2:
                    vector.wait_ge(s_out, 32 * (st - 1))
                wmv = ap(wm_buf)[:, st % 2, :]
                vector.tensor_tensor(wmv, ap(w_buf)[:, st % 2, :],
                                     tri_v[:, st, :], op=mybir.AluOpType.mult)
                vector.tensor_scalar(ap(mf_buf)[:], wmv, THRESH, None,
                                     op0=mybir.AluOpType.is_gt)
                vector.tensor_tensor(wmv, wmv, ap(mf_buf)[:],
                                     op=mybir.AluOpType.mult)
                vector.tensor_copy(ap(m8_buf)[:, st % 2, :],
                                   ap(mf_buf)[:]).then_inc(d_ep, 1)

            vector.wait_ge(s_in, 16)   # misc (acol)
            next_epi = 0
            for (k, st, chk, b, e) in sched:
                if e == "v":
                    if first_chunk_use[("v", chk)] == k:
                        vector.wait_ge(s_in, 32 + 16 * (chk + 1))
                    if k >= NRING:
                        vector.wait_ge(pe, k - NRING + 1)
                    vector.tensor_scalar(
                        ap(ring)[:, k % NRING, :],
                        ap(brep_sb)[:, chk, min(st, 3), :],
                        acol_v[:, st, b, :][:, chk:chk + 1], 0.0,
                        op0=mybir.AluOpType.add,
                        op1=mybir.AluOpType.max).then_inc(g_dve, 1)
                while next_epi < NST and k >= last_mm[next_epi] + 16:
                    v_epilogue(next_epi)
                    next_epi += 1
            while next_epi < NST:
                v_epilogue(next_epi)
                next_epi += 1

        @block.scalar
        def _(scalar):
            def a_epilogue(st):
                scalar.wait_ge(pe, last_mm[st])
                if st >= 2:
                    scalar.wait_ge(d_ep, st - 1)   # w_buf slot free (DVE)
                scalar.activation(ap(w_buf)[:, st % 2, :], ap(ps[st])[:, :],
                                  mybir.ActivationFunctionType.Sigmoid,
                                  bias=b2_v, scale=1.0).then_inc(a_ep, 1)

            next_epi = 0
            for (k, st, chk, b, e) in sched:
                if e == "a":
                    if first_chunk_use[("a", chk)] == k:
                        scalar.wait_ge(s_in, 32 + 16 * (chk + 1))
                    if k >= NRING:
                        scalar.wait_ge(pe, k - NRING + 1)
                    scalar.activation(
                        ap(ring)[:, k % NRING, :],
                        ap(brep_sb)[:, chk, min(st, 3), :],
                        mybir.ActivationFunctionType.Relu,
                        bias=acol_v[:, st, b, :][:, chk:chk + 1],
                        scale=1.0).then_inc(g_act, 1)
                    while next_epi < NST and k >= last_mm[next_epi] + 8:
                        a_epilogue(next_epi)
                        next_epi += 1
            while next_epi < NST:
                a_epilogue(next_epi)
                next_epi += 1

    return nc


def kernel(node_features, node_emb, W_fe, b_fe, W1, b1, W2, b2):
    import concourse.bass as bass
    import concourse.mybir as mybir
    from concourse.bass_utils import run_bass_kernel_spmd

    nf = np.asarray(node_features, np.float32)
    emb = np.asarray(node_emb, np.float32)
    W_fe = np.asarray(W_fe, np.float32)
    b_fe = np.asarray(b_fe, np.float32)
    W1 = np.asarray(W1, np.float32)
    b1 = np.asarray(b1, np.float32)
    W2v = np.asarray(W2, np.float32)[0]
    b2v = float(np.asarray(b2, np.float32)[0])

    comb = nf @ W_fe.T + b_fe + emb
    absw = np.abs(W2v)
    sgn = np.sign(W2v).astype(np.float32)
    A = (comb @ W1[:, :H].T * absw).astype(np.float32)
    BT = ((comb @ W1[:, H:].T + b1) * absw).astype(np.float32).T.copy()

    nc = _build_graph(bass, mybir)
    in_maps = [_build_core_inputs(c, A, BT, sgn, b2v) for c in range(NCORES)]
    res = run_bass_kernel_spmd(nc, in_maps, core_ids=list(range(NCORES)))

    full_w = np.zeros((N, N), np.float32)
    full_m = np.zeros((N, N), bool)
    for c in range(NCORES):
        i_top, i_bot, sts = _core_layout(c)
        wo = res.results[c]["w_out"].reshape(NST, P, JC).astype(np.float32)
        mo = res.results[c]["m_out"].reshape(NST, P, JC)
        for st, (blk, ch_j) in enumerate(sts):
            iblk = i_top if blk == 0 else i_bot
            j0 = 512 * ch_j
            full_w[iblk:iblk + P, j0:j0 + JC] = wo[st]
            full_m[iblk:iblk + P, j0:j0 + JC] = mo[st].astype(bool)
    return full_w, full_m


# revision 6
# speedup vs baseline: 1.3668x; 1.2668x over previous
"""AdaptiveGraphLearning kernel for 8 TRN2 NeuronCores (v2a, fp16 gen+matmul).

w[i,j] = sigmoid(sum_h W2[h]*relu(A[i,h]+B[j,h]) + b2), strict upper triangle,
thresholded at 0.1. Only the upper triangle is computed (2x saving) with a
balanced row split: core c owns row blocks [128c,128c+128) and
[2048-128(c+1), 2048-128c) -- exactly 5 [128,512] output supertiles per core.

v2a design:
 - A,B prescaled by |W2| on host so the reduction selector S is +-1 and the
   W2 multiply costs nothing on device; logit = sum_h sign(W2h)*relu(A'+B').
 - fp16 brep/ring/S: DVE tensor_scalar runs in 2x/4x perf mode, matmul
   moving data is fp16 (1 col/cycle), input DMA is ~5MB instead of 16MB.
 - G=8 packing: K=128 packs 8 i's x 16 h's, so brep is only replicated 8x.
 - st-outer schedule: each supertile's PSUM finishes right after its own
   64 matmuls, so sigmoid/threshold/mask/DMA overlap the next supertile;
   epilogues are issued a few gen ops into the next supertile so the gen
   engines don't stall on the PE ring drain.
 - epilogue ops and semaphore style are the baseline's (per-op incs).
"""
import numpy as np

N, F, H = 2048, 256, 64
P, JC = 128, 512
G = 8                 # i's per matmul band
HC = 16               # h's per chunk
NCH = H // HC         # 4 chunks
NB = P // G           # 16 col-groups
NCORES = 8
NST = 5               # supertiles per core
NSLOT = 4             # brep j-span slots (st 3,4 share slot 3)
NRING = 24            # R-tile ring size
NMM = NST * NCH * NB  # 320 matmuls / gen ops
THRESH = 0.1


def _core_layout(c):
    """Return (i_top, i_bot, sts) where sts is a list of (block, chunk)."""
    i_top = 128 * c
    i_bot = N - 128 * (c + 1)
    tops = [(0, j) for j in range(i_top // 512, 4)]
    bots = [(1, j) for j in range(i_bot // 512, 4)]
    sts = [x for x in tops if x != (0, 3)] + [x for x in bots if x != (1, 3)]
    sts = sts + [(0, 3), (1, 3)]
    assert len(sts) == NST, (c, sts)
    return i_top, i_bot, sts


def _schedule():
    """Global op order: st outer, then chunk, then band. Returns list of
    (k, st, chk, b, eng) with eng in {'v','a'}."""
    sched = []
    for st in range(NST):
        for chk in range(NCH):
            for b in range(NB):
                k = len(sched)
                sched.append((k, st, chk, b, "a" if k % 10 in (2, 5, 8) else "v"))
    return sched


def _build_core_inputs(c, A, BT, sgn, b2):
    """Host-side layout prep for one core (pure reformatting of A'/B'/sign).

    A is prescaled A*|W2| [N,H] f32; BT is prescaled (B*|W2|).T [H,N] f32;
    sgn is sign(W2) [H]."""
    i_top, i_bot, sts = _core_layout(c)
    pg = np.arange(P) % G          # g  = p % 8   -> i offset within band
    ph = np.arange(P) // G         # hc = p // 8  -> h within chunk

    # brep[p, ch, slot, :] = BT[HC*ch + p//G, jspan(slot)]   (fp16)
    brep = np.empty((P, NCH, NSLOT, JC), np.float16)
    for s in range(NSLOT):
        blk, ch_j = sts[s]
        j0 = 512 * ch_j
        for chk in range(NCH):
            brep[:, chk, s, :] = BT[HC * chk + ph][:, j0:j0 + JC]

    # acol[p, st, b, ch] = A[iblk(st) + G*b + p%G, HC*ch + p//G]   (f32)
    acol = np.empty((P, NST, NB, NCH), np.float32)
    tri = np.empty((P, NST, JC), np.float32)
    for st, (blk, ch_j) in enumerate(sts):
        iblk = i_top if blk == 0 else i_bot
        j0 = 512 * ch_j
        for b in range(NB):
            rows = iblk + G * b + pg           # [128]
            cols = HC * np.arange(NCH)[None, :] + ph[:, None]   # [128, NCH]
            acol[:, st, b, :] = A[rows[:, None], cols]
        jj = j0 + np.arange(JC)[None, :]
        ii = (iblk + np.arange(P))[:, None]
        tri[:, st, :] = (jj > ii).astype(np.float32)

    # shifted-window selector: s_pad[p, chk, G*(NB-1)+p%G] = sign(W2[h]);
    # lhsT for (chk,b) = s_pad[:, chk, G*(NB-1-b) : +128]
    PAD = P + G * (NB - 1)
    s_pad = np.zeros((P, NCH, PAD), np.float16)
    for chk in range(NCH):
        s_pad[np.arange(P), chk, G * (NB - 1) + pg] = sgn[HC * chk + ph]

    b2bc = np.full((P, 1), b2, np.float32)
    misc = np.concatenate(
        [acol.reshape(P, -1), tri.reshape(P, -1), b2bc],
        axis=1).astype(np.float32).copy()
    return {"brep": brep.reshape(P, -1).copy(),
            "aux": s_pad.reshape(P, -1).copy(), "misc": misc}


def _build_graph(bass, mybir):
    nc = bass.Bass()
    MF_ACOL = NST * NB * NCH
    MF_TRI = NST * JC
    MF = MF_ACOL + MF_TRI + 1
    PAD = P + G * (NB - 1)
    AF = NCH * PAD

    brep_e = nc.declare_dram_parameter("brep", [P, NCH * NSLOT * JC], mybir.dt.float16, isOutput=False)
    aux_e = nc.declare_dram_parameter("aux", [P, AF], mybir.dt.float16, isOutput=False)
    misc_e = nc.declare_dram_parameter("misc", [P, MF], mybir.dt.float32, isOutput=False)
    w_e = nc.declare_dram_parameter("w_out", [NST * P, JC], mybir.dt.float32, isOutput=True)
    m_e = nc.declare_dram_parameter("m_out", [NST * P, JC], mybir.dt.uint8, isOutput=True)

    brep_sb = nc.alloc_sbuf_tensor("brep_sb", [P, NCH, NSLOT, JC], mybir.dt.float16)
    aux_sb = nc.alloc_sbuf_tensor("aux_sb", [P, AF], mybir.dt.float16)
    misc_sb = nc.alloc_sbuf_tensor("misc_sb", [P, MF], mybir.dt.float32)
    ring = nc.alloc_sbuf_tensor("ring", [P, NRING, JC], mybir.dt.float16)
    w_buf = nc.alloc_sbuf_tensor("w_buf", [P, 2, JC], mybir.dt.float32)
    wm_buf = nc.alloc_sbuf_tensor("wm_buf", [P, 2, JC], mybir.dt.float32)
    mf_buf = nc.alloc_sbuf_tensor("mf_buf", [P, JC], mybir.dt.float32)
    m8_buf = nc.alloc_sbuf_tensor("m8_buf", [P, 2, JC], mybir.dt.uint8)
    ps = [nc.alloc_psum_tensor(f"ps{st}", [P, JC], mybir.dt.float32) for st in range(NST)]

    def ap(h):
        return h.ap() if hasattr(h, "ap") else h

    misc = ap(misc_sb)
    acol_v = misc[:, :MF_ACOL].rearrange("p (st b ch) -> p st b ch", st=NST, b=NB)
    tri_v = misc[:, MF_ACOL:MF_ACOL + MF_TRI].rearrange(
        "p (st j) -> p st j", st=NST)
    b2_v = misc[:, MF - 1:MF]
    spad_v = ap(aux_sb)[:, :].rearrange("p (ch v) -> p ch v", ch=NCH)

    sched = _schedule()
    ndve_le = np.cumsum([1 if e == "v" else 0 for (_, _, _, _, e) in sched])
    nact_le = np.cumsum([1 if e == "a" else 0 for (_, _, _, _, e) in sched])
    last_mm = {st: 64 * (st + 1) for st in range(NST)}   # pe value when ps[st] done

    first_chunk_use = {}
    for (k, st, chk, b, e) in sched:
        key = (e, chk)
        if key not in first_chunk_use:
            first_chunk_use[key] = k

    with (nc.Block() as block,
          nc.semaphore("s_in") as s_in,
          nc.semaphore("g_dve") as g_dve,
          nc.semaphore("g_act") as g_act,
          nc.semaphore("pe") as pe,
          nc.semaphore("a_ep") as a_ep,
          nc.semaphore("d_ep") as d_ep,
          nc.semaphore("s_out") as s_out):

        @block.sync
        def _(sync):
            bs = ap(brep_sb)
            be = ap(brep_e).rearrange("p (ch r) -> p ch r", ch=NCH)
            sync.dma_start(out=misc[:], in_=ap(misc_e)).then_inc(s_in, 16)
            sync.dma_start(out=bs[:, 0], in_=be[:, 0]).then_inc(s_in, 16)
            sync.dma_start(out=ap(aux_sb)[:, :], in_=ap(aux_e)).then_inc(s_in, 16)
            for chk in range(1, NCH):
                sync.dma_start(out=bs[:, chk], in_=be[:, chk]).then_inc(s_in, 16)
            for st in range(NST):
                sync.wait_ge(d_ep, st + 1)
                sync.dma_start(out=ap(w_e)[st * P:(st + 1) * P, :],
                               in_=ap(wm_buf)[:, st % 2, :]).then_inc(s_out, 16)
                sync.dma_start(out=ap(m_e)[st * P:(st + 1) * P, :],
                               in_=ap(m8_buf)[:, st % 2, :]).then_inc(s_out, 16)
            sync.wait_ge(s_out, 32 * NST)

        @block.tensor
        def _(tensor):
            tensor.wait_ge(s_in, 48)   # aux (s_pad + acol) loaded
            for (k, st, chk, b, e) in sched:
                if e == "v":
                    tensor.wait_ge(g_dve, int(ndve_le[k]))
                else:
                    tensor.wait_ge(g_act, int(nact_le[k]))
                tensor.matmul(ap(ps[st])[:, :],
                              lhsT=spad_v[:, chk, G * (NB - 1 - b):G * (NB - 1 - b) + P],
                              rhs=ap(ring)[:, k % NRING, :],
                              start=(chk == 0 and b == 0),
                              stop=(chk == NCH - 1 and b == NB - 1),
                              skip_group_check=True).then_inc(pe, 1)

        @block.vector
        def _(vector):
            def v_epilogue(st):
                # issued ~16 gen ops into the next supertile so the PE ring
                # drain has already completed and this doesn't stall gen
                vector.wait_ge(a_ep, st + 1)
                if st >= 2:
                    vector.wait_ge(s_out, 32 * (st - 1))
                wmv = ap(wm_buf)[:, st % 2, :]
                vector.tensor_tensor(wmv, ap(w_buf)[:, st % 2, :],
                                     tri_v[:, st, :], op=mybir.AluOpType.mult)
                vector.tensor_scalar(ap(mf_buf)[:], wmv, THRESH, None,
                                     op0=mybir.AluOpType.is_gt)
                vector.tensor_tensor(wmv, wmv, ap(mf_buf)[:],
                                     op=mybir.AluOpType.mult)
                vector.tensor_copy(ap(m8_buf)[:, st % 2, :],
                                   ap(mf_buf)[:]).then_inc(d_ep, 1)

            vector.wait_ge(s_in, 48)   # misc + brep0 + aux
            next_epi = 0
            for (k, st, chk, b, e) in sched:
                if e == "v":
                    if first_chunk_use[("v", chk)] == k:
                        vector.wait_ge(s_in, 32 if chk == 0 else 48 + 16 * chk)
                    if k >= NRING:
                        vector.wait_ge(pe, k - NRING + 1)
                    vector.tensor_scalar(
                        ap(ring)[:, k % NRING, :],
                        ap(brep_sb)[:, chk, min(st, 3), :],
                        acol_v[:, st, b, :][:, chk:chk + 1], 0.0,
                        op0=mybir.AluOpType.add,
                        op1=mybir.AluOpType.max).then_inc(g_dve, 1)
                while next_epi < NST and k >= last_mm[next_epi] + 16:
                    v_epilogue(next_epi)
                    next_epi += 1
            while next_epi < NST:
                v_epilogue(next_epi)
                next_epi += 1

        @block.scalar
        def _(scalar):
            def a_epilogue(st):
                scalar.wait_ge(pe, last_mm[st])
                if st >= 2:
                    scalar.wait_ge(d_ep, st - 1)   # w_buf slot free (DVE)
                scalar.activation(ap(w_buf)[:, st % 2, :], ap(ps[st])[:, :],
                                  mybir.ActivationFunctionType.Sigmoid,
                                  bias=b2_v, scale=1.0).then_inc(a_ep, 1)

            next_epi = 0
            for (k, st, chk, b, e) in sched:
                if e == "a":
                    if first_chunk_use[("a", chk)] == k:
                        scalar.wait_ge(s_in, 32 if chk == 0 else 48 + 16 * chk)
                    if k >= NRING:
                        scalar.wait_ge(pe, k - NRING + 1)
                    scalar.activation(
                        ap(ring)[:, k % NRING, :],
                        ap(brep_sb)[:, chk, min(st, 3), :],
                        mybir.ActivationFunctionType.Relu,
                        bias=acol_v[:, st, b, :][:, chk:chk + 1],
                        scale=1.0).then_inc(g_act, 1)
                    while next_epi < NST and k >= last_mm[next_epi] + 8:
                        a_epilogue(next_epi)
                        next_epi += 1
            while next_epi < NST:
                a_epilogue(next_epi)
                next_epi += 1

    return nc


def kernel(node_features, node_emb, W_fe, b_fe, W1, b1, W2, b2):
    import concourse.bass as bass
    import concourse.mybir as mybir
    from concourse.bass_utils import run_bass_kernel_spmd

    nf = np.asarray(node_features, np.float32)
    emb = np.asarray(node_emb, np.float32)
    W_fe = np.asarray(W_fe, np.float32)
    b_fe = np.asarray(b_fe, np.float32)
    W1 = np.asarray(W1, np.float32)
    b1 = np.asarray(b1, np.float32)
    W2v = np.asarray(W2, np.float32)[0]
    b2v = float(np.asarray(b2, np.float32)[0])

    comb = nf @ W_fe.T + b_fe + emb
    absw = np.abs(W2v)
    sgn = np.sign(W2v).astype(np.float32)
    A = (comb @ W1[:, :H].T * absw).astype(np.float32)
    BT = ((comb @ W1[:, H:].T + b1) * absw).astype(np.float32).T.copy()

    nc = _build_graph(bass, mybir)
    in_maps = [_build_core_inputs(c, A, BT, sgn, b2v) for c in range(NCORES)]
    res = run_bass_kernel_spmd(nc, in_maps, core_ids=list(range(NCORES)))

    full_w = np.zeros((N, N), np.float32)
    full_m = np.zeros((N, N), bool)
    for c in range(NCORES):
        i_top, i_bot, sts = _core_layout(c)
        wo = res.results[c]["w_out"].reshape(NST, P, JC).astype(np.float32)
        mo = res.results[c]["m_out"].reshape(NST, P, JC)
        for st, (blk, ch_j) in enumerate(sts):
            iblk = i_top if blk == 0 else i_bot
            j0 = 512 * ch_j
            full_w[iblk:iblk + P, j0:j0 + JC] = wo[st]
            full_m[iblk:iblk + P, j0:j0 + JC] = mo[st].astype(bool)
    return full_w, full_m


# revision 7
# speedup vs baseline: 1.4699x; 1.0754x over previous
"""AdaptiveGraphLearning kernel for 8 TRN2 NeuronCores (v3, fp16 + trapezoid).

w[i,j] = sigmoid(sum_h W2[h]*relu(A[i,h]+B[j,h]) + b2), strict upper triangle,
thresholded at 0.1.

Design:
 - Upper triangle only, trapezoid tiling: core c owns row blocks
   [128c, 128c+128) and [128(15-c), +128). Each block's j-window starts AT
   its diagonal (128-aligned), not 512-aligned, so almost no wasted columns.
   Combined span is 2176 cols for every core; unified tile slots
   [512,512,512,512,256] (max 128 pad cols, masked by tri) keep the SPMD
   graph identical across cores. The narrow tile runs last -> short tail.
 - A,B prescaled by |W2| on host so the reduction selector is +-1 and the
   W2 multiply is free; logit = sum_h sign(W2h)*relu(A'+B').
 - fp16 brep/ring/selector: matmul moving data at 1 col/cycle @2.4GHz,
   DVE tensor_scalar in 16-bit perf mode, small input DMA.
 - G=8 packing: K=128 packs 8 i's x 16 h's (brep replicated only 8x).
 - shifted-window selector s_pad: lhsT for (chk,b) is a 128-col window of
   a [P, NCH, 248] fp16 tensor -> 254KB instead of 2MB.
 - st-outer schedule: each tile's PSUM finishes right after its own 64
   matmuls; sigmoid/threshold/mask/DMA overlap the next tile's compute
   (epilogues issued a few gen ops into the next tile).
"""
import numpy as np

N, F, H = 2048, 256, 64
P = 128
G = 8                 # i's per matmul band
HC = 16               # h's per chunk
NCH = H // HC         # 4 chunks
NB = P // G           # 16 col-groups
NCORES = 8
NST = 5               # tiles per core
SLOTW = [512, 512, 512, 512, 256]     # unified tile slot widths
SLOT_OFF = [0, 512, 1024, 1536, 2048]
JCT = sum(SLOTW)      # 2304
NRING = 24            # R-tile ring size (slots are 512 wide)
NMM = NST * NCH * NB  # 320 matmuls / gen ops
THRESH = 0.1


def _core_layout(c):
    """Return list of NST tiles (iblk, j0, w) sorted to match SLOTW.

    Core c owns row blocks r1=c and r2=15-c; each block's column window is
    [128*r, 2048), cut into 512-wide tiles plus one remainder."""
    tiles = []
    for r in (c, 15 - c):
        iblk = 128 * r
        j = iblk
        while j < N:
            w = min(512, N - j)
            tiles.append((iblk, j, w))
            j += w
    tiles.sort(key=lambda t: -t[2])
    assert len(tiles) == NST, (c, tiles)
    for st, (iblk, j0, w) in enumerate(tiles):
        assert w <= SLOTW[st], (c, st, tiles)
    return tiles


def _schedule():
    """Global op order: st outer, then chunk, then band."""
    sched = []
    for st in range(NST):
        for chk in range(NCH):
            for b in range(NB):
                k = len(sched)
                sched.append((k, st, chk, b, "a" if k % 10 in (2, 5, 8) else "v"))
    return sched


def _build_core_inputs(c, A, BT, sgn, b2):
    """Host-side layout prep for one core (pure reformatting of A'/B'/sign).

    A is prescaled A*|W2| [N,H] f32; BT is prescaled (B*|W2|).T [H,N] f32;
    sgn is sign(W2) [H]."""
    tiles = _core_layout(c)
    pg = np.arange(P) % G          # g  = p % 8   -> i offset within band
    ph = np.arange(P) // G         # hc = p // 8  -> h within chunk

    # brep[p, ch, SLOT_OFF[st]:+w] = BT[HC*ch + p//G, j0:j0+w]  (fp16, pad 0)
    brep = np.zeros((P, NCH, JCT), np.float16)
    acol = np.empty((P, NST, NB, NCH), np.float32)
    tri = np.zeros((P, JCT), np.float32)
    for st, (iblk, j0, w) in enumerate(tiles):
        off = SLOT_OFF[st]
        for chk in range(NCH):
            brep[:, chk, off:off + w] = BT[HC * chk + ph][:, j0:j0 + w]
        for b in range(NB):
            rows = iblk + G * b + pg           # [128]
            cols = HC * np.arange(NCH)[None, :] + ph[:, None]   # [128, NCH]
            acol[:, st, b, :] = A[rows[:, None], cols]
        jj = j0 + np.arange(w)[None, :]
        ii = (iblk + np.arange(P))[:, None]
        tri[:, off:off + w] = (jj > ii).astype(np.float32)

    # shifted-window selector: s_pad[p, chk, G*(NB-1)+p%G] = sign(W2[h]);
    # lhsT for (chk,b) = s_pad[:, chk, G*(NB-1-b) : +128]
    PAD = P + G * (NB - 1)
    s_pad = np.zeros((P, NCH, PAD), np.float16)
    for chk in range(NCH):
        s_pad[np.arange(P), chk, G * (NB - 1) + pg] = sgn[HC * chk + ph]

    b2bc = np.full((P, 1), b2, np.float32)
    misc = np.concatenate(
        [acol.reshape(P, -1), tri, b2bc], axis=1).astype(np.float32).copy()
    return {"brep": brep.reshape(P, -1).copy(),
            "aux": s_pad.reshape(P, -1).copy(), "misc": misc}


def _build_graph(bass, mybir):
    nc = bass.Bass()
    MF_ACOL = NST * NB * NCH
    MF = MF_ACOL + JCT + 1
    PAD = P + G * (NB - 1)
    AF = NCH * PAD

    brep_e = nc.declare_dram_parameter("brep", [P, NCH * JCT], mybir.dt.float16, isOutput=False)
    aux_e = nc.declare_dram_parameter("aux", [P, AF], mybir.dt.float16, isOutput=False)
    misc_e = nc.declare_dram_parameter("misc", [P, MF], mybir.dt.float32, isOutput=False)
    w_e = nc.declare_dram_parameter("w_out", [NST * P, 512], mybir.dt.float32, isOutput=True)
    m_e = nc.declare_dram_parameter("m_out", [NST * P, 512], mybir.dt.uint8, isOutput=True)

    brep_sb = nc.alloc_sbuf_tensor("brep_sb", [P, NCH, JCT], mybir.dt.float16)
    aux_sb = nc.alloc_sbuf_tensor("aux_sb", [P, AF], mybir.dt.float16)
    misc_sb = nc.alloc_sbuf_tensor("misc_sb", [P, MF], mybir.dt.float32)
    ring = nc.alloc_sbuf_tensor("ring", [P, NRING, 512], mybir.dt.float16)
    w_buf = nc.alloc_sbuf_tensor("w_buf", [P, 2, 512], mybir.dt.float32)
    wm_buf = nc.alloc_sbuf_tensor("wm_buf", [P, 2, 512], mybir.dt.float32)
    mf_buf = nc.alloc_sbuf_tensor("mf_buf", [P, 512], mybir.dt.float32)
    m8_buf = nc.alloc_sbuf_tensor("m8_buf", [P, 2, 512], mybir.dt.uint8)
    ps = [nc.alloc_psum_tensor(f"ps{st}", [P, 512], mybir.dt.float32) for st in range(NST)]

    def ap(h):
        return h.ap() if hasattr(h, "ap") else h

    misc = ap(misc_sb)
    acol_v = misc[:, :MF_ACOL].rearrange("p (st b ch) -> p st b ch", st=NST, b=NB)
    tri_v = misc[:, MF_ACOL:MF_ACOL + JCT]
    b2_v = misc[:, MF - 1:MF]
    spad_v = ap(aux_sb)[:, :].rearrange("p (ch v) -> p ch v", ch=NCH)

    sched = _schedule()
    ndve_le = np.cumsum([1 if e == "v" else 0 for (_, _, _, _, e) in sched])
    nact_le = np.cumsum([1 if e == "a" else 0 for (_, _, _, _, e) in sched])
    last_mm = {st: 64 * (st + 1) for st in range(NST)}   # pe value when ps[st] done

    first_chunk_use = {}
    for (k, st, chk, b, e) in sched:
        key = (e, chk)
        if key not in first_chunk_use:
            first_chunk_use[key] = k

    with (nc.Block() as block,
          nc.semaphore("s_in") as s_in,
          nc.semaphore("g_dve") as g_dve,
          nc.semaphore("g_act") as g_act,
          nc.semaphore("pe") as pe,
          nc.semaphore("a_ep") as a_ep,
          nc.semaphore("d_ep") as d_ep,
          nc.semaphore("s_out") as s_out):

        @block.sync
        def _(sync):
            bs = ap(brep_sb)
            be = ap(brep_e).rearrange("p (ch r) -> p ch r", ch=NCH)
            sync.dma_start(out=misc[:], in_=ap(misc_e)).then_inc(s_in, 16)
            sync.dma_start(out=bs[:, 0], in_=be[:, 0]).then_inc(s_in, 16)
            sync.dma_start(out=ap(aux_sb)[:, :], in_=ap(aux_e)).then_inc(s_in, 16)
            for chk in range(1, NCH):
                sync.dma_start(out=bs[:, chk], in_=be[:, chk]).then_inc(s_in, 16)
            for st in range(NST):
                w = SLOTW[st]
                sync.wait_ge(d_ep, st + 1)
                sync.dma_start(out=ap(w_e)[st * P:(st + 1) * P, :w],
                               in_=ap(wm_buf)[:, st % 2, :w]).then_inc(s_out, 16)
                sync.dma_start(out=ap(m_e)[st * P:(st + 1) * P, :w],
                               in_=ap(m8_buf)[:, st % 2, :w]).then_inc(s_out, 16)
            sync.wait_ge(s_out, 32 * NST)

        @block.tensor
        def _(tensor):
            tensor.wait_ge(s_in, 48)   # aux (s_pad) loaded
            for (k, st, chk, b, e) in sched:
                w = SLOTW[st]
                if e == "v":
                    tensor.wait_ge(g_dve, int(ndve_le[k]))
                else:
                    tensor.wait_ge(g_act, int(nact_le[k]))
                tensor.matmul(ap(ps[st])[:, :w],
                              lhsT=spad_v[:, chk, G * (NB - 1 - b):G * (NB - 1 - b) + P],
                              rhs=ap(ring)[:, k % NRING, :w],
                              start=(chk == 0 and b == 0),
                              stop=(chk == NCH - 1 and b == NB - 1),
                              skip_group_check=True).then_inc(pe, 1)

        @block.vector
        def _(vector):
            def v_epilogue(st):
                # issued ~16 gen ops into the next tile so the PE ring
                # drain has already completed and this doesn't stall gen
                w = SLOTW[st]
                off = SLOT_OFF[st]
                vector.wait_ge(a_ep, st + 1)
                if st >= 2:
                    vector.wait_ge(s_out, 32 * (st - 1))
                wmv = ap(wm_buf)[:, st % 2, :w]
                vector.tensor_tensor(wmv, ap(w_buf)[:, st % 2, :w],
                                     tri_v[:, off:off + w],
                                     op=mybir.AluOpType.mult)
                vector.tensor_scalar(ap(mf_buf)[:, :w], wmv, THRESH, None,
                                     op0=mybir.AluOpType.is_gt)
                vector.tensor_tensor(wmv, wmv, ap(mf_buf)[:, :w],
                                     op=mybir.AluOpType.mult)
                vector.tensor_copy(ap(m8_buf)[:, st % 2, :w],
                                   ap(mf_buf)[:, :w]).then_inc(d_ep, 1)

            vector.wait_ge(s_in, 48)   # misc + brep0 + aux
            next_epi = 0
            for (k, st, chk, b, e) in sched:
                if e == "v":
                    w = SLOTW[st]
                    if first_chunk_use[("v", chk)] == k:
                        vector.wait_ge(s_in, 32 if chk == 0 else 48 + 16 * chk)
                    if k >= NRING:
                        vector.wait_ge(pe, k - NRING + 1)
                    vector.tensor_scalar(
                        ap(ring)[:, k % NRING, :w],
                        ap(brep_sb)[:, chk, SLOT_OFF[st]:SLOT_OFF[st] + w],
                        acol_v[:, st, b, :][:, chk:chk + 1], 0.0,
                        op0=mybir.AluOpType.add,
                        op1=mybir.AluOpType.max).then_inc(g_dve, 1)
                while next_epi < NST and k >= last_mm[next_epi] + 16:
                    v_epilogue(next_epi)
                    next_epi += 1
            while next_epi < NST:
                v_epilogue(next_epi)
                next_epi += 1

        @block.scalar
        def _(scalar):
            def a_epilogue(st):
                w = SLOTW[st]
                scalar.wait_ge(pe, last_mm[st])
                if st >= 2:
                    scalar.wait_ge(d_ep, st - 1)   # w_buf slot free (DVE)
                scalar.activation(ap(w_buf)[:, st % 2, :w], ap(ps[st])[:, :w],
                                  mybir.ActivationFunctionType.Sigmoid,
                                  bias=b2_v, scale=1.0).then_inc(a_ep, 1)

            next_epi = 0
            for (k, st, chk, b, e) in sched:
                if e == "a":
                    w = SLOTW[st]
                    if first_chunk_use[("a", chk)] == k:
                        scalar.wait_ge(s_in, 32 if chk == 0 else 48 + 16 * chk)
                    if k >= NRING:
                        scalar.wait_ge(pe, k - NRING + 1)
                    scalar.activation(
                        ap(ring)[:, k % NRING, :w],
                        ap(brep_sb)[:, chk, SLOT_OFF[st]:SLOT_OFF[st] + w],
                        mybir.ActivationFunctionType.Relu,
                        bias=acol_v[:, st, b, :][:, chk:chk + 1],
                        scale=1.0).then_inc(g_act, 1)
                    while next_epi < NST and k >= last_mm[next_epi] + 8:
                        a_epilogue(next_epi)
                        next_epi += 1
            while next_epi < NST:
                a_epilogue(next_epi)
                next_epi += 1

    return nc


def kernel(node_features, node_emb, W_fe, b_fe, W1, b1, W2, b2):
    import concourse.bass as bass
    import concourse.mybir as mybir
    from concourse.bass_utils import run_bass_kernel_spmd

    nf = np.asarray(node_features, np.float32)
    emb = np.asarray(node_emb, np.float32)
    W_fe = np.asarray(W_fe, np.float32)
    b_fe = np.asarray(b_fe, np.float32)
    W1 = np.asarray(W1, np.float32)
    b1 = np.asarray(b1, np.float32)
    W2v = np.asarray(W2, np.float32)[0]
    b2v = float(np.asarray(b2, np.float32)[0])

    comb = nf @ W_fe.T + b_fe + emb
    absw = np.abs(W2v)
    sgn = np.sign(W2v).astype(np.float32)
    A = (comb @ W1[:, :H].T * absw).astype(np.float32)
    BT = ((comb @ W1[:, H:].T + b1) * absw).astype(np.float32).T.copy()

    nc = _build_graph(bass, mybir)
    in_maps = [_build_core_inputs(c, A, BT, sgn, b2v) for c in range(NCORES)]
    res = run_bass_kernel_spmd(nc, in_maps, core_ids=list(range(NCORES)))

    full_w = np.zeros((N, N), np.float32)
    full_m = np.zeros((N, N), bool)
    for c in range(NCORES):
        tiles = _core_layout(c)
        wo = res.results[c]["w_out"].reshape(NST, P, 512)
        mo = res.results[c]["m_out"].reshape(NST, P, 512)
        for st, (iblk, j0, w) in enumerate(tiles):
            full_w[iblk:iblk + P, j0:j0 + w] = wo[st][:, :w]
            full_m[iblk:iblk + P, j0:j0 + w] = mo[st][:, :w].astype(bool)
    return full_w, full_m
